# revision 1
# baseline (speedup 1.0000x reference)
"""Multi-head causal self-attention (B=4, T=2048, D=1024, H=16) on 8 TRN2 cores.

Sharding (hardcoded): data-parallel over the 4 batches x tensor-parallel over
head halves. Core c handles batch c//2 and local heads (c%2)*8 .. (c%2)*8+7
for all 2048 positions. Every core runs the same SPMD program on its slice:

  x[b] [2048,1024] -> x^T in SBUF via PE-transpose (bf16 cast on psum copy)
  Q^T = (Wq_slice)^T x^T / 8,  K^T = (Wk_slice)^T x^T       [dh-pairs packed
  V   = x Wv_slice (+ ones column for the softmax denominator)   on 128 parts]
  S^T = K Q^T per 128x512 block (four query blocks share one matmul; causal
        blocks only; head pairs run concurrently via PE row groups; future
        positions get -1e9 via triangular/full-mask matmul accumulands),
  P^T = exp(S^T) (ScalarE, straight from PSUM, bf16 out),
  ctx^T = V^T P (V is the stationary operand, so ctx comes out transposed;
        row 64 of the product is the softmax denominator l),
  ctx^T *= 1/l (reciprocal + partition-broadcast DMA + multiply),
  partial_out = ctx^T.T @ Wo_slice.

The host sums the two partial outputs per batch and adds the bias bo.
"""
import numpy as np

import concourse.bass as bass
import concourse.mybir as mybir
import concourse.tile as tile
from concourse import bacc
from concourse.bass_utils import run_bass_kernel_spmd
from concourse.masks import make_upper_triangular

F32 = mybir.dt.float32
BF16 = mybir.dt.bfloat16
AF = mybir.ActivationFunctionType

B, T, D = 4, 2048, 1024
HL = 8              # local heads per core
HP = HL // 2        # local head pairs (two heads share 128 partitions)
DH = 64
PO = D // 128       # contraction chunks over D
CD = HL * DH        # 512: local context feature dim
FC = CD // 128      # 4
NB = T // 128       # 16 query/key blocks of 128
QUAD = 4            # query blocks handled together (512 S^T columns)
SCALE = 1.0 / 8.0   # 1/sqrt(DH)
NEG = -1e9
CHUNK = 2           # key blocks per S^T psum tile ([128, 2*512] = 2 banks)


def _emit_attention_quad(nc, qb0, kt_sb, qt_sb, v_sb, utri01,
                         ptp, stp, cxp, lvp, ctp, dramp, ctxt16s):
    """Attention for query blocks qb0..qb0+3, all 4 local head pairs.

    For each key block kb one N=512 matmul covers all four query blocks.
    Future (k > q) positions are pushed to -1e9 by accumulating mask
    matmuls (identity^T @ utri / negf), so exp() zeroes them and they
    drop out of both the context and the denominator.
    """
    nkb = qb0 + QUAD   # key blocks needed (for the last query block)
    qsl = slice(qb0 * 128, (qb0 + QUAD) * 128)
    for hp in range(HP):
        pt_e = ptp.tile([128, NB, 512], BF16, tag="pt")
        pt_o = ptp.tile([128, NB, 512], BF16, tag="pt")
        nchunks = (nkb + CHUNK - 1) // CHUNK
        for ch in range(nchunks):
            k0 = ch * CHUNK
            k1 = min(nkb, k0 + CHUNK)
            st_e = stp.tile([128, 512 * CHUNK], F32, tag="st")
            st_o = stp.tile([128, 512 * CHUNK], F32, tag="st")
            for kb in range(k0, k1):
                w = (kb - k0) * 512
                for st, lo in ((st_e, 0), (st_o, 64)):
                    nc.tensor.matmul(
                        st[:, w:w + 512],
                        lhsT=kt_sb[lo:lo + 64, hp, kb * 128:(kb + 1) * 128],
                        rhs=qt_sb[lo:lo + 64, hp, qsl],
                        start=True, stop=True,
                    )
            ncol = (k1 - k0) * 512
            nc.scalar.activation(pt_e[:, k0:k1, :], st_e[:, :ncol], AF.Exp)
            nc.scalar.activation(pt_o[:, k0:k1, :], st_o[:, :ncol], AF.Exp)
            # causal masking on P^T (VectorE): zero the columns of query
            # blocks left of the diagonal and the strict upper triangle of
            # the diagonal block
            for kb in range(max(k0, qb0), k1):
                j = kb - qb0
                for pt in (pt_e, pt_o):
                    if j > 0:
                        nc.vector.memset(pt[:, kb, :j * 128], 0.0)
                    nc.vector.tensor_mul(
                        pt[:, kb, j * 128:(j + 1) * 128],
                        pt[:, kb, j * 128:(j + 1) * 128], utri01)
        # AV as one tight accumulation burst per parity
        ctxT = {}
        for par, pt in ((0, pt_e), (1, pt_o)):
            ctxT[par] = cxp.tile([128, 512], F32, tag="cx", name=f"ctxT{par}")
            for kb in range(nkb):
                nc.tensor.matmul(
                    ctxT[par][0:65, :],
                    lhsT=v_sb[:, kb, 2 * hp + par, :],
                    rhs=pt[:, kb, :],
                    start=(kb == 0), stop=(kb == nkb - 1),
                )
        # Copy ctx^T out of PSUM immediately (frees the banks for the next
        # head pair), then normalize lazily: ctx^T[dh, q] /= l[q].
        # Both parities' l rows are gathered into one [64,16] tile via
        # cross-partition SBUF DMAs, inverted cheaply, bounced through DRAM
        # once for the partition broadcast, and divided in.
        ctxt16 = ctxt16s[hp]
        ctxu = {}
        lv2 = lvp.tile([64, 16], BF16, tag="lv2")
        for par in (0, 1):
            ctxu[par] = lvp.tile([65, 512], BF16, tag="ctxu",
                                 name=f"ctxu{par}")
            nc.vector.tensor_copy(ctxu[par], ctxT[par][0:65, :])
            nc.sync.dma_start(lv2[:, par * 8:(par + 1) * 8],
                              ctxu[par][64:65, :])
        lv2i = lvp.tile([64, 16], F32, tag="lv2i")
        nc.vector.reciprocal(lv2i, lv2)
        # store parity-major/q-contiguous: element (p, par, c) -> par*512+p*8+c
        ldram = dramp.tile([2, 512], F32, tag="ldram")
        store = bass.AP(tensor=ldram.tensor, offset=ldram.offset,
                        ap=[[8, 64], [512, 2], [1, 8]])
        nc.sync.dma_start(store, lv2i.rearrange("p (a c) -> p a c", c=8))
        lb = lvp.tile([64, 2, 512], F32, tag="lb")
        bcast = bass.AP(tensor=ldram.tensor, offset=ldram.offset,
                        ap=[[0, 64], [512, 2], [1, 512]])
        nc.sync.dma_start(lb, bcast)
        nc.vector.tensor_mul(ctxt16[0:64, :], ctxu[0][0:64, :], lb[:, 0, :])
        tmp = ctp.tile([64, 512], BF16, tag="ctmp")
        nc.vector.tensor_mul(tmp, ctxu[1][0:64, :], lb[:, 1, :])
        nc.sync.dma_start(ctxt16[64:128, :], tmp)


def _emit_out_proj_qb(nc, qb0, qloc, ctxt16s, wo_sb, mmp, osbp, out_d):
    """Output projection for query block qb0 + qloc."""
    qb = qb0 + qloc
    for dw in range(2):
        ps = mmp.tile([128, 512], F32, tag="mm")
        for hp in range(HP):
            nc.tensor.matmul(
                ps, lhsT=ctxt16s[hp][:, qloc * 128:(qloc + 1) * 128],
                rhs=wo_sb[:, hp, dw * 512:(dw + 1) * 512],
                start=(hp == 0), stop=(hp == HP - 1),
            )
        osb = osbp.tile([128, 512], F32, tag="osb")
        nc.vector.tensor_copy(osb, ps)
        nc.sync.dma_start(out_d[qb * 128:(qb + 1) * 128, dw * 512:(dw + 1) * 512], osb)


def _emit_xt_tb(nc, tb, x_d, xb16_d, xstage, x16p):
    """One 128-row block of x: f32 load, bf16 cast, store to DRAM scratch."""
    xf = xstage.tile([128, D], F32, tag="xf")
    nc.scalar.dma_start(xf, x_d[tb * 128:(tb + 1) * 128, :])
    x16 = x16p.tile([128, D], BF16, tag="x16")
    if tb % 2 == 0:
        nc.vector.tensor_copy(x16, xf)
    else:
        nc.scalar.activation(x16, xf, AF.Copy)
    nc.sync.dma_start(xb16_d[tb * 128:(tb + 1) * 128, :], x16)


def _emit_proj_tw(nc, tw, xt_sb, mmp, wq_sb, wk_sb, wv_sb, kt_sb, qt_sb, v_sb):
    """Project K^T, Q^T, V for one 512-column group of x^T."""
    tsl = slice(tw * 512, (tw + 1) * 512)
    for hp in range(HP):
        ps = mmp.tile([128, 512], F32, tag="mm")
        for po in range(PO):
            nc.tensor.matmul(
                ps, lhsT=wk_sb[:, po, hp * 128:(hp + 1) * 128], rhs=xt_sb[:, po, tsl],
                start=(po == 0), stop=(po == PO - 1),
            )
        nc.vector.tensor_copy(kt_sb[:, hp, tsl], ps)
    for hp in range(HP):
        ps = mmp.tile([128, 512], F32, tag="mm")
        for po in range(PO):
            nc.tensor.matmul(
                ps, lhsT=wq_sb[:, po, hp * 128:(hp + 1) * 128], rhs=xt_sb[:, po, tsl],
                start=(po == 0), stop=(po == PO - 1),
            )
        nc.scalar.activation(qt_sb[:, hp, tsl], ps, AF.Copy, scale=SCALE)
    for tb in range(4):
        kb = tw * 4 + tb
        ps = mmp.tile([128, 512], F32, tag="mm")
        for po in range(PO):
            nc.tensor.matmul(
                ps, lhsT=xt_sb[:, po, kb * 128:(kb + 1) * 128], rhs=wv_sb[:, po, :],
                start=(po == 0), stop=(po == PO - 1),
            )
        nc.vector.tensor_copy(
            v_sb[:, kb, :, 0:64], ps.rearrange("p (h d) -> p h d", h=HL)
        )


def build_nc():
    nc = bacc.Bacc("TRN2", target_bir_lowering=False)
    x_d = nc.dram_tensor("x", [T, D], F32, kind="ExternalInput")
    wq_d = nc.dram_tensor("wq", [D, CD], F32, kind="ExternalInput")
    wk_d = nc.dram_tensor("wk", [D, CD], F32, kind="ExternalInput")
    wv_d = nc.dram_tensor("wv", [D, CD], F32, kind="ExternalInput")
    wo_d = nc.dram_tensor("wo", [CD, D], F32, kind="ExternalInput")
    out_d = nc.dram_tensor("out", [T, D], F32, kind="ExternalOutput")
    xb16_d = nc.dram_tensor("xb16", [T, D], BF16)  # internal scratch

    with tile.TileContext(nc) as tc:
        with (
            tc.tile_pool(name="consts", bufs=1) as consts,
            tc.tile_pool(name="wsb", bufs=1) as wsb,
            tc.tile_pool(name="wstage", bufs=1) as wstage,
            tc.tile_pool(name="xstage", bufs=2) as xstage,
            tc.tile_pool(name="x16", bufs=2) as x16p,
            tc.tile_pool(name="big", bufs=1) as big,
            tc.tile_pool(name="pt", bufs=2) as ptp,
            tc.tile_pool(name="lv", bufs=2) as lvp,
            tc.tile_pool(name="ct", bufs=2) as ctp,
            tc.tile_pool(name="ctxt16", bufs=4) as ctxt16p,
            tc.tile_pool(name="osb", bufs=2) as osbp,
            tc.tile_pool(name="dram", bufs=4, space="DRAM") as dramp,
            tc.tile_pool(name="mm", bufs=2, space="PSUM") as mmp,
            tc.tile_pool(name="st", bufs=2, space="PSUM") as stp,
            tc.tile_pool(name="cx", bufs=2, space="PSUM") as cxp,
        ):
            utri01 = consts.tile([128, 128], BF16, tag="utri01")
            make_upper_triangular(nc, utri01, val=1.0, diag=True)

            wq_sb = wsb.tile([128, PO, CD], BF16, tag="wq")
            wk_sb = wsb.tile([128, PO, CD], BF16, tag="wk")
            wv_sb = wsb.tile([128, PO, CD], BF16, tag="wv")
            wo_sb = wsb.tile([128, FC, D], BF16, tag="wo")
            for i, (dram, sb, shp) in enumerate((
                (wq_d, wq_sb, (PO, CD)),
                (wk_d, wk_sb, (PO, CD)),
                (wv_d, wv_sb, (PO, CD)),
                (wo_d, wo_sb, (FC, D)),
            )):
                stg = wstage.tile([128, shp[0], shp[1]], F32, tag="ws")
                nc.sync.dma_start(stg, dram.rearrange("(po p) n -> p po n", p=128))
                nc.vector.tensor_copy(sb, stg)

            xt_sb = big.tile([128, PO, T], BF16, tag="xt")
            kt_sb = big.tile([128, HP, T], BF16, tag="kt")
            qt_sb = big.tile([128, HP, T], BF16, tag="qt")
            v_sb = big.tile([128, NB, HL, 65], BF16, tag="v")
            nc.gpsimd.memset(v_sb[:, :, :, 64:65], 1.0)

            for half in range(2):
                for tw in range(2 * half, 2 * half + 2):
                    # x -> bf16 -> DRAM scratch, then per-group DMA
                    # transposes feeding this group's projections
                    for tb4 in range(4):
                        _emit_xt_tb(nc, tw * 4 + tb4, x_d, xb16_d, xstage,
                                    x16p)
                    for po in range(PO):
                        nc.sync.dma_start_transpose(
                            xt_sb[:, po, tw * 512:(tw + 1) * 512],
                            xb16_d[tw * 512:(tw + 1) * 512,
                                   po * 128:(po + 1) * 128])
                    _emit_proj_tw(nc, tw, xt_sb, mmp,
                                  wq_sb, wk_sb, wv_sb, kt_sb, qt_sb, v_sb)
                # deeper quad first: its long AV/normalize tail overlaps the
                # shallower quad's S^T phase
                for qb0 in (8 * half + QUAD, 8 * half):
                    ctxt16s = [ctxt16p.tile([128, 512], BF16, tag="c16",
                                            name=f"c16_{hp}")
                               for hp in range(HP)]
                    _emit_attention_quad(nc, qb0, kt_sb, qt_sb, v_sb, utri01,
                                         ptp, stp, cxp, lvp, ctp,
                                         dramp, ctxt16s)
                    for qloc in range(QUAD):
                        _emit_out_proj_qb(nc, qb0, qloc, ctxt16s, wo_sb,
                                          mmp, osbp, out_d)

    nc.compile()
    return nc


_CACHE = {}


def _get_nc():
    if "nc" not in _CACHE:
        _CACHE["nc"] = build_nc()
    return _CACHE["nc"]


def make_in_maps(x, Wq, Wk, Wv, Wo):
    x = np.asarray(x, np.float32)
    Wq = np.asarray(Wq, np.float32)
    Wk = np.asarray(Wk, np.float32)
    Wv = np.asarray(Wv, np.float32)
    Wo = np.asarray(Wo, np.float32)
    in_maps = []
    for c in range(8):
        b, hh = c // 2, c % 2
        cols = slice(hh * CD, (hh + 1) * CD)
        in_maps.append({
            "x": np.ascontiguousarray(x[b]),
            "wq": np.ascontiguousarray(Wq[:, cols]),
            "wk": np.ascontiguousarray(Wk[:, cols]),
            "wv": np.ascontiguousarray(Wv[:, cols]),
            "wo": np.ascontiguousarray(Wo[cols, :]),
        })
    return in_maps


def gather_output(results, bo):
    bo = np.asarray(bo, np.float32)
    out = np.empty((B, T, D), np.float32)
    for b in range(B):
        out[b] = results[2 * b]["out"] + results[2 * b + 1]["out"] + bo[None, :]
    return out


def kernel(x, Wq, Wk, Wv, Wo, bo):
    nc = _get_nc()
    in_maps = make_in_maps(x, Wq, Wk, Wv, Wo)
    res = run_bass_kernel_spmd(nc, in_maps, core_ids=list(range(8)))
    return gather_output(res.results, bo)



# revision 10
# speedup vs baseline: 1.2018x; 1.2018x over previous
"""Multi-head causal self-attention (B=4, T=2048, D=1024, H=16) on 8 TRN2 cores.

Sharding (hardcoded): data-parallel over the 4 batches x tensor-parallel over
head halves. Core c handles batch c//2 and local heads (c%2)*8 .. (c%2)*8+7
for all 2048 positions. The host sums the two partial outputs per batch and
adds the bias bo.

Per-core schedule (v2): projections and attention are interleaved so the
tensor engine never starves and the scalar engine (exp) pipelines underneath:

  for g in 0..3:   x positions g*512..g*512+511, query quad qb0 = 4g
    x[g] -> bf16 -> DRAM -> DMA-transpose -> xt                       (DMA)
    K^T/Q^T/V projections for this position group                     (PE)
    attention quad g (needs only projections <= g):
      per head pair, per key block kb (causal-trimmed columns):
        S^T both parities -> one PSUM tile [128, 2, 512]              (PE)
        future positions masked by accumulating a strict-upper -1e9
        triangular matmul on the diagonal block                       (PE)
        P^T = exp(S^T * 1/8)  (scale folded into the activation)      (ACT)
        ctx^T[par] += V[kb,par] (+ones column) @ P^T[par]             (PE)
      normalize: l row -> reciprocal -> gpsimd partition-broadcast
        -> multiply; parity 1 shifted to partitions 64..127 via DMA
    out projection per query block + DMA out (direct from PSUM)
"""
import numpy as np

import concourse.bass as bass
import concourse.mybir as mybir
import concourse.tile as tile
from concourse import bacc
from concourse.bass_utils import run_bass_kernel_spmd
from concourse.masks import make_identity, make_upper_triangular

F32 = mybir.dt.float32
BF16 = mybir.dt.bfloat16
AF = mybir.ActivationFunctionType

B, T, D = 4, 2048, 1024
HL = 8              # local heads per core
HP = HL // 2        # local head pairs (two heads share 128 partitions)
DH = 64
PO = D // 128       # contraction chunks over D
CD = HL * DH        # 512: local context feature dim
FC = CD // 128      # 4
NB = T // 128       # 16 query/key blocks of 128
QUAD = 4            # query blocks handled together (512 S^T columns)
SCALE = 1.0 / 8.0   # 1/sqrt(DH)
NEG = -1e9


def _emit_x_group(nc, g, x_d, xb16_d, xt_sb, xstage, x16p):
    """x rows g*512..g*512+511: f32 load, bf16 cast, DRAM bounce, transpose."""
    for tb4 in range(4):
        tb = g * 4 + tb4
        xf = xstage.tile([128, D], F32, tag="xf")
        nc.scalar.dma_start(xf, x_d[tb * 128:(tb + 1) * 128, :])
        x16 = x16p.tile([128, D], BF16, tag="x16")
        nc.vector.tensor_copy(x16, xf)
        nc.gpsimd.dma_start(xb16_d[tb * 128:(tb + 1) * 128, :], x16)
    for po in range(PO):
        nc.sync.dma_start_transpose(
            xt_sb[:, po, g * 512:(g + 1) * 512],
            xb16_d[g * 512:(g + 1) * 512, po * 128:(po + 1) * 128])


def _emit_proj_tw(nc, tw, xt_sb, mmp, wq_sb, wk_sb, wv_sb, kt_sb, qt_sb, v_sb):
    """Project K^T, Q^T, V for one 512-column group of x^T."""
    tsl = slice(tw * 512, (tw + 1) * 512)
    for hp in range(HP):
        ps = mmp.tile([128, 512], F32, tag="mm")
        for po in range(PO):
            nc.tensor.matmul(
                ps, lhsT=wk_sb[:, po, hp * 128:(hp + 1) * 128], rhs=xt_sb[:, po, tsl],
                start=(po == 0), stop=(po == PO - 1),
            )
        nc.vector.tensor_copy(kt_sb[:, hp, tsl], ps)
    for hp in range(HP):
        ps = mmp.tile([128, 512], F32, tag="mm")
        for po in range(PO):
            nc.tensor.matmul(
                ps, lhsT=wq_sb[:, po, hp * 128:(hp + 1) * 128], rhs=xt_sb[:, po, tsl],
                start=(po == 0), stop=(po == PO - 1),
            )
        nc.vector.tensor_copy(qt_sb[:, hp, tsl], ps)
    for tb in range(4):
        kb = tw * 4 + tb
        ps = mmp.tile([128, 512], F32, tag="mm")
        for po in range(PO):
            nc.tensor.matmul(
                ps, lhsT=xt_sb[:, po, kb * 128:(kb + 1) * 128], rhs=wv_sb[:, po, :],
                start=(po == 0), stop=(po == PO - 1),
            )
        nc.vector.tensor_copy(
            v_sb[:, kb, :, 0:64], ps.rearrange("p (h d) -> p h d", h=HL)
        )


def _emit_quad(nc, qb0, kt_sb, qt_sb, v_sb, utri_neg, ident,
               stp, ptp, cxp, lvp, ctxt16p, dramp):
    """Attention for query blocks qb0..qb0+3, all 4 local head pairs.

    Chunk-level pipeline per key block: S^T (both parities into one PSUM
    tile) -> exp -> AV accumulation, so the scalar engine runs one block
    behind the tensor engine. Columns left of the diagonal are never
    computed or consumed (causal trim); the diagonal block gets -1e9 on
    its strict upper triangle via an accumulated mask matmul.
    """
    nkb = qb0 + QUAD
    ctxt16s = []
    for hp in range(HP):
        ctx = [cxp.tile([65, 512], F32, tag="ctx", name=f"ctx{par}")
               for par in (0, 1)]
        for kb in range(nkb):
            j = kb - qb0
            c0 = max(j, 0) * 128   # first live column in the 512-q window
            st = stp.tile([128, 2, 512], F32, tag="st")
            for par, lo in ((0, 0), (1, 64)):
                nc.tensor.matmul(
                    st[:, par, c0:512],
                    lhsT=kt_sb[lo:lo + 64, hp, kb * 128:(kb + 1) * 128],
                    rhs=qt_sb[lo:lo + 64, hp, qb0 * 128 + c0:(qb0 + QUAD) * 128],
                    start=True, stop=(j < 0),
                    skip_group_check=(j >= 0),
                )
            if j >= 0:
                for par in (0, 1):
                    nc.tensor.matmul(
                        st[:, par, c0:c0 + 128], lhsT=utri_neg, rhs=ident,
                        start=False, stop=True, skip_group_check=True,
                    )
            pt = ptp.tile([128, 2, 512], BF16, tag="pt")
            nc.scalar.activation(pt[:, :, c0:512], st[:, :, c0:512],
                                 AF.Exp, scale=SCALE)
            for par in (0, 1):
                nc.tensor.matmul(
                    ctx[par][:, c0:512],
                    lhsT=v_sb[:, kb, 2 * hp + par, :],
                    rhs=pt[:, par, c0:512],
                    start=(kb == 0), stop=(kb == nkb - 1),
                    skip_group_check=True,
                )
        # normalize: ctx^T[dh, q] /= l[q] with l in row 64 (baseline-proven
        # path: gather l rows into [64,16], reciprocal, DRAM bounce for the
        # partition broadcast, multiply; parity 1 shifted via SBUF DMA).
        ctxu = {}
        lv2 = lvp.tile([64, 16], BF16, tag="lv2")
        for par in (0, 1):
            ctxu[par] = lvp.tile([65, 512], BF16, tag="ctxu",
                                 name=f"ctxu{par}")
            nc.vector.tensor_copy(ctxu[par], ctx[par])
            nc.sync.dma_start(lv2[:, par * 8:(par + 1) * 8],
                              ctxu[par][64:65, :])
        lv2i = lvp.tile([64, 16], F32, tag="lv2i")
        nc.vector.reciprocal(lv2i, lv2)
        # store parity-major/q-contiguous: element (p, par, c) -> par*512+p*8+c
        ldram = dramp.tile([2, 512], F32, tag="ldram")
        store = bass.AP(tensor=ldram.tensor, offset=ldram.offset,
                        ap=[[8, 64], [512, 2], [1, 8]])
        nc.sync.dma_start(store, lv2i.rearrange("p (a c) -> p a c", c=8))
        lb = lvp.tile([64, 2, 512], F32, tag="lb")
        bcast = bass.AP(tensor=ldram.tensor, offset=ldram.offset,
                        ap=[[0, 64], [512, 2], [1, 512]])
        nc.sync.dma_start(lb, bcast)
        c16 = ctxt16p.tile([128, 512], BF16, tag="c16", name=f"c16_{hp}")
        nc.vector.tensor_mul(c16[0:64, :], ctxu[0][0:64, :], lb[:, 0, :])
        tmp = lvp.tile([64, 512], BF16, tag="ctmp")
        nc.vector.tensor_mul(tmp, ctxu[1][0:64, :], lb[:, 1, :])
        nc.sync.dma_start(c16[64:128, :], tmp)
        ctxt16s.append(c16)
    return ctxt16s


def _emit_out_proj_qb(nc, qb0, qloc, ctxt16s, wo_sb, mmp, osbp, out_d):
    """Output projection for query block qb0 + qloc."""
    qb = qb0 + qloc
    for dw in range(2):
        ps = mmp.tile([128, 512], F32, tag="mm")
        for hp in range(HP):
            nc.tensor.matmul(
                ps, lhsT=ctxt16s[hp][:, qloc * 128:(qloc + 1) * 128],
                rhs=wo_sb[:, hp, dw * 512:(dw + 1) * 512],
                start=(hp == 0), stop=(hp == HP - 1),
            )
        osb = osbp.tile([128, 512], F32, tag="osb")
        if dw == 0:
            nc.vector.tensor_copy(osb, ps)
        else:
            nc.scalar.activation(osb, ps, AF.Copy)
        nc.scalar.dma_start(
            out_d[qb * 128:(qb + 1) * 128, dw * 512:(dw + 1) * 512], osb)


def build_nc():
    nc = bacc.Bacc("TRN2", target_bir_lowering=False)
    x_d = nc.dram_tensor("x", [T, D], F32, kind="ExternalInput")
    wq_d = nc.dram_tensor("wq", [D, CD], F32, kind="ExternalInput")
    wk_d = nc.dram_tensor("wk", [D, CD], F32, kind="ExternalInput")
    wv_d = nc.dram_tensor("wv", [D, CD], F32, kind="ExternalInput")
    wo_d = nc.dram_tensor("wo", [CD, D], F32, kind="ExternalInput")
    out_d = nc.dram_tensor("out", [T, D], F32, kind="ExternalOutput")
    xb16_d = nc.dram_tensor("xb16", [T, D], BF16)  # internal scratch

    with tile.TileContext(nc) as tc:
        with (
            tc.tile_pool(name="consts", bufs=1) as consts,
            tc.tile_pool(name="wsb", bufs=1) as wsb,
            tc.tile_pool(name="wstage", bufs=2) as wstage,
            tc.tile_pool(name="xstage", bufs=2) as xstage,
            tc.tile_pool(name="x16", bufs=2) as x16p,
            tc.tile_pool(name="big", bufs=1) as big,
            tc.tile_pool(name="pt", bufs=4) as ptp,
            tc.tile_pool(name="lv", bufs=2) as lvp,
            tc.tile_pool(name="ctxt16", bufs=4) as ctxt16p,
            tc.tile_pool(name="osb", bufs=2) as osbp,
            tc.tile_pool(name="dram", bufs=4, space="DRAM") as dramp,
            tc.tile_pool(name="mm", bufs=2, space="PSUM") as mmp,
            tc.tile_pool(name="st", bufs=2, space="PSUM") as stp,
            tc.tile_pool(name="cx", bufs=2, space="PSUM") as cxp,
        ):
            utri_neg = consts.tile([128, 128], BF16, tag="utri_neg")
            make_upper_triangular(nc, utri_neg, val=NEG, diag=False)
            ident = consts.tile([128, 128], BF16, tag="ident")
            make_identity(nc, ident)

            wq_sb = wsb.tile([128, PO, CD], BF16, tag="wq")
            wk_sb = wsb.tile([128, PO, CD], BF16, tag="wk")
            wv_sb = wsb.tile([128, PO, CD], BF16, tag="wv")
            wo_sb = wsb.tile([128, FC, D], BF16, tag="wo")
            # two staging slots; loads ride two queues, casts two engines
            for i, (dram, sb, shp) in enumerate((
                (wk_d, wk_sb, (PO, CD)),
                (wq_d, wq_sb, (PO, CD)),
                (wv_d, wv_sb, (PO, CD)),
                (wo_d, wo_sb, (FC, D)),
            )):
                stg = wstage.tile([128, shp[0], shp[1]], F32, tag="ws")
                eng = nc.scalar if i % 2 == 0 else nc.gpsimd
                eng.dma_start(stg, dram.rearrange("(po p) n -> p po n", p=128))
                ceng = nc.vector if i % 2 == 0 else nc.scalar
                if ceng is nc.scalar:
                    nc.scalar.activation(sb, stg, AF.Copy)
                else:
                    nc.vector.tensor_copy(sb, stg)

            xt_sb = big.tile([128, PO, T], BF16, tag="xt")
            kt_sb = big.tile([128, HP, T], BF16, tag="kt")
            qt_sb = big.tile([128, HP, T], BF16, tag="qt")
            v_sb = big.tile([128, NB, HL, 65], BF16, tag="v")
            nc.gpsimd.memset(v_sb[:, :, :, 64:65], 1.0)

            # interleaved emission: projections run one group ahead of
            # attention so dense PE work always exists while exp pipelines
            for g in range(4):
                _emit_x_group(nc, g, x_d, xb16_d, xt_sb, xstage, x16p)
                _emit_proj_tw(nc, g, xt_sb, mmp,
                              wq_sb, wk_sb, wv_sb, kt_sb, qt_sb, v_sb)
                if g >= 1:
                    qb0 = (g - 1) * QUAD
                    ctxt16s = _emit_quad(nc, qb0, kt_sb, qt_sb, v_sb,
                                         utri_neg, ident,
                                         stp, ptp, cxp, lvp, ctxt16p, dramp)
                    for qloc in range(QUAD):
                        _emit_out_proj_qb(nc, qb0, qloc, ctxt16s, wo_sb,
                                          mmp, osbp, out_d)
            qb0 = 3 * QUAD
            ctxt16s = _emit_quad(nc, qb0, kt_sb, qt_sb, v_sb,
                                 utri_neg, ident,
                                 stp, ptp, cxp, lvp, ctxt16p, dramp)
            for qloc in range(QUAD):
                _emit_out_proj_qb(nc, qb0, qloc, ctxt16s, wo_sb, mmp, osbp,
                                  out_d)

    nc.compile()
    return nc


_CACHE = {}


def _get_nc():
    if "nc" not in _CACHE:
        _CACHE["nc"] = build_nc()
    return _CACHE["nc"]


def make_in_maps(x, Wq, Wk, Wv, Wo):
    x = np.asarray(x, np.float32)
    Wq = np.asarray(Wq, np.float32)
    Wk = np.asarray(Wk, np.float32)
    Wv = np.asarray(Wv, np.float32)
    Wo = np.asarray(Wo, np.float32)
    in_maps = []
    for c in range(8):
        b, hh = c // 2, c % 2
        cols = slice(hh * CD, (hh + 1) * CD)
        in_maps.append({
            "x": np.ascontiguousarray(x[b]),
            "wq": np.ascontiguousarray(Wq[:, cols]),
            "wk": np.ascontiguousarray(Wk[:, cols]),
            "wv": np.ascontiguousarray(Wv[:, cols]),
            "wo": np.ascontiguousarray(Wo[cols, :]),
        })
    return in_maps


def gather_output(results, bo):
    bo = np.asarray(bo, np.float32)
    out = np.empty((B, T, D), np.float32)
    for b in range(B):
        out[b] = results[2 * b]["out"] + results[2 * b + 1]["out"] + bo[None, :]
    return out


def kernel(x, Wq, Wk, Wv, Wo, bo):
    nc = _get_nc()
    in_maps = make_in_maps(x, Wq, Wk, Wv, Wo)
    res = run_bass_kernel_spmd(nc, in_maps, core_ids=list(range(8)))
    return gather_output(res.results, bo)


# revision 12
# speedup vs baseline: 1.2091x; 1.0060x over previous
"""Multi-head causal self-attention (B=4, T=2048, D=1024, H=16) on 8 TRN2 cores.

Sharding (hardcoded): data-parallel over the 4 batches x tensor-parallel over
head halves. Core c handles batch c//2 and local heads (c%2)*8 .. (c%2)*8+7
for all 2048 positions. The host casts x and the weight slices to bf16 and
pre-packs weights into the on-chip layout (one contiguous 8KB row per
partition); it sums the two partial outputs per batch and adds the bias bo.

Per-core schedule: projections and attention interleave so the tensor engine
never starves while the scalar engine (exp) pipelines underneath:

  for g in 0..3:   x positions g*512..g*512+511, query quad qb0 = 4g
    8 DMA-transposes x_d -> xt columns for this group                 (DMA)
    K^T/Q^T/V projections for this position group                     (PE)
    attention quad g (needs only projections <= g):
      per head pair, per key block kb (causal-trimmed columns):
        S^T both parities -> one PSUM tile [128, 2, 512]; the two
        64-contraction matmuls run concurrently in PE row groups      (PE)
        future positions get -1e9 via an accumulated strict-upper
        triangular mask matmul on the diagonal block                  (PE)
        P^T = exp(S^T * 1/8)  (scale folded into the activation)      (ACT)
        ctx^T[par] += V[kb,par] (+ones column) @ P^T[par]             (PE)
      normalize: l rows -> [64,16] reciprocal -> DRAM-bounce
        partition-broadcast -> multiply; parity 1 shifted via DMA
    out projection per query block
"""
import numpy as np
import ml_dtypes

import concourse.bass as bass
import concourse.mybir as mybir
import concourse.tile as tile
from concourse import bacc
from concourse.bass_utils import run_bass_kernel_spmd
from concourse.masks import make_identity, make_upper_triangular

F32 = mybir.dt.float32
BF16 = mybir.dt.bfloat16
AF = mybir.ActivationFunctionType
BF16NP = ml_dtypes.bfloat16

B, T, D = 4, 2048, 1024
HL = 8              # local heads per core
HP = HL // 2        # local head pairs (two heads share 128 partitions)
DH = 64
PO = D // 128       # contraction chunks over D
CD = HL * DH        # 512: local context feature dim
FC = CD // 128      # 4
NB = T // 128       # 16 query/key blocks of 128
QUAD = 4            # query blocks handled together (512 S^T columns)
SCALE = 1.0 / 8.0   # 1/sqrt(DH)
NEG = -1e9


def _emit_proj_tw(nc, tw, xt_sb, mmp, wq_sb, wk_sb, wv_sb, kt_sb, qt_sb, v_sb):
    """Project K^T, Q^T, V for one 512-column group of x^T."""
    tsl = slice(tw * 512, (tw + 1) * 512)
    for hp in range(HP):
        ps = mmp.tile([128, 512], F32, tag="mm")
        for po in range(PO):
            nc.tensor.matmul(
                ps, lhsT=wk_sb[:, po, hp * 128:(hp + 1) * 128], rhs=xt_sb[:, po, tsl],
                start=(po == 0), stop=(po == PO - 1),
            )
        nc.scalar.activation(kt_sb[:, hp, tsl], ps, AF.Copy)
    for hp in range(HP):
        ps = mmp.tile([128, 512], F32, tag="mm")
        for po in range(PO):
            nc.tensor.matmul(
                ps, lhsT=wq_sb[:, po, hp * 128:(hp + 1) * 128], rhs=xt_sb[:, po, tsl],
                start=(po == 0), stop=(po == PO - 1),
            )
        nc.vector.tensor_copy(qt_sb[:, hp, tsl], ps)
    for tb in range(4):
        kb = tw * 4 + tb
        ps = mmp.tile([128, 512], F32, tag="mm")
        for po in range(PO):
            nc.tensor.matmul(
                ps, lhsT=xt_sb[:, po, kb * 128:(kb + 1) * 128], rhs=wv_sb[:, po, :],
                start=(po == 0), stop=(po == PO - 1),
            )
        nc.vector.tensor_copy(
            v_sb[:, kb, :, 0:64], ps.rearrange("p (h d) -> p h d", h=HL)
        )


def _emit_quad(nc, qb0, kt_sb, qt_sb, v_sb, utri_neg, ident,
               stp, ptp, cxp, lvp, ctxt16p, dramp):
    """Attention for query blocks qb0..qb0+3, all 4 local head pairs.

    Chunk-level pipeline per key block: S^T (both parities into one PSUM
    tile) -> exp -> AV accumulation, so the scalar engine runs one block
    behind the tensor engine. Columns left of the diagonal are never
    computed or consumed (causal trim); the diagonal block gets -1e9 on
    its strict upper triangle via an accumulated mask matmul.
    """
    nkb = qb0 + QUAD
    ctxt16s = []
    for hp in range(HP):
        ctx = [cxp.tile([65, 512], F32, tag="ctx", name=f"ctx{par}")
               for par in (0, 1)]
        for kb in range(nkb):
            j = kb - qb0
            c0 = max(j, 0) * 128   # first live column in the 512-q window
            st = stp.tile([128, 2, 512], F32, tag="st")
            for par, lo in ((0, 0), (1, 64)):
                nc.tensor.matmul(
                    st[:, par, c0:512],
                    lhsT=kt_sb[lo:lo + 64, hp, kb * 128:(kb + 1) * 128],
                    rhs=qt_sb[lo:lo + 64, hp, qb0 * 128 + c0:(qb0 + QUAD) * 128],
                    start=True, stop=(j < 0),
                    skip_group_check=(j >= 0),
                )
            if j >= 0:
                for par in (0, 1):
                    nc.tensor.matmul(
                        st[:, par, c0:c0 + 128], lhsT=utri_neg, rhs=ident,
                        start=False, stop=True, skip_group_check=True,
                    )
            pt = ptp.tile([128, 2, 512], BF16, tag="pt")
            nc.scalar.activation(pt[:, :, c0:512], st[:, :, c0:512],
                                 AF.Exp, scale=SCALE)
            for par in (0, 1):
                nc.tensor.matmul(
                    ctx[par][:, c0:512],
                    lhsT=v_sb[:, kb, 2 * hp + par, :],
                    rhs=pt[:, par, c0:512],
                    start=(kb == 0), stop=(kb == nkb - 1),
                    skip_group_check=True,
                )
        # normalize: ctx^T[dh, q] /= l[q] with l in row 64 (baseline-proven
        # path: gather l rows into [64,16], reciprocal, DRAM bounce for the
        # partition broadcast, multiply; parity 1 shifted via SBUF DMA).
        ctxu = {}
        lv2 = lvp.tile([64, 16], BF16, tag="lv2")
        for par in (0, 1):
            ctxu[par] = lvp.tile([65, 512], BF16, tag="ctxu",
                                 name=f"ctxu{par}")
            nc.vector.tensor_copy(ctxu[par], ctx[par])
            nc.sync.dma_start(lv2[:, par * 8:(par + 1) * 8],
                              ctxu[par][64:65, :])
        lv2i = lvp.tile([64, 16], F32, tag="lv2i")
        nc.vector.reciprocal(lv2i, lv2)
        # store parity-major/q-contiguous: element (p, par, c) -> par*512+p*8+c
        ldram = dramp.tile([2, 512], F32, tag="ldram")
        store = bass.AP(tensor=ldram.tensor, offset=ldram.offset,
                        ap=[[8, 64], [512, 2], [1, 8]])
        nc.sync.dma_start(store, lv2i.rearrange("p (a c) -> p a c", c=8))
        lb = lvp.tile([64, 2, 512], F32, tag="lb")
        bcast = bass.AP(tensor=ldram.tensor, offset=ldram.offset,
                        ap=[[0, 64], [512, 2], [1, 512]])
        nc.sync.dma_start(lb, bcast)
        c16 = ctxt16p.tile([128, 512], BF16, tag="c16", name=f"c16_{hp}")
        nc.vector.tensor_mul(c16[0:64, :], ctxu[0][0:64, :], lb[:, 0, :])
        tmp = lvp.tile([64, 512], BF16, tag="ctmp")
        nc.vector.tensor_mul(tmp, ctxu[1][0:64, :], lb[:, 1, :])
        nc.sync.dma_start(c16[64:128, :], tmp)
        ctxt16s.append(c16)
    return ctxt16s


def _emit_out_proj_qb(nc, qb0, qloc, ctxt16s, wo_sb, mmp, osbp, out_d):
    """Output projection for query block qb0 + qloc."""
    qb = qb0 + qloc
    for dw in range(2):
        ps = mmp.tile([128, 512], F32, tag="mm")
        for hp in range(HP):
            nc.tensor.matmul(
                ps, lhsT=ctxt16s[hp][:, qloc * 128:(qloc + 1) * 128],
                rhs=wo_sb[:, hp, dw * 512:(dw + 1) * 512],
                start=(hp == 0), stop=(hp == HP - 1),
            )
        osb = osbp.tile([128, 512], F32, tag="osb")
        nc.vector.tensor_copy(osb, ps)
        nc.scalar.dma_start(
            out_d[qb * 128:(qb + 1) * 128, dw * 512:(dw + 1) * 512], osb)


def build_nc():
    nc = bacc.Bacc("TRN2", target_bir_lowering=False)
    x_d = nc.dram_tensor("x", [T, D], BF16, kind="ExternalInput")
    wq_d = nc.dram_tensor("wq", [128, PO * CD], BF16, kind="ExternalInput")
    wk_d = nc.dram_tensor("wk", [128, PO * CD], BF16, kind="ExternalInput")
    wv_d = nc.dram_tensor("wv", [128, PO * CD], BF16, kind="ExternalInput")
    wo_d = nc.dram_tensor("wo", [128, FC * D], BF16, kind="ExternalInput")
    out_d = nc.dram_tensor("out", [T, D], F32, kind="ExternalOutput")

    with tile.TileContext(nc) as tc:
        with (
            tc.tile_pool(name="consts", bufs=1) as consts,
            tc.tile_pool(name="wsb", bufs=1) as wsb,
            tc.tile_pool(name="big", bufs=1) as big,
            tc.tile_pool(name="pt", bufs=6) as ptp,
            tc.tile_pool(name="lv", bufs=2) as lvp,
            tc.tile_pool(name="ctxt16", bufs=4) as ctxt16p,
            tc.tile_pool(name="osb", bufs=2) as osbp,
            tc.tile_pool(name="dram", bufs=4, space="DRAM") as dramp,
            tc.tile_pool(name="mm", bufs=2, space="PSUM") as mmp,
            tc.tile_pool(name="st", bufs=2, space="PSUM") as stp,
            tc.tile_pool(name="cx", bufs=2, space="PSUM") as cxp,
        ):
            utri_neg = consts.tile([128, 128], BF16, tag="utri_neg")
            make_upper_triangular(nc, utri_neg, val=NEG, diag=False)
            ident = consts.tile([128, 128], BF16, tag="ident")
            make_identity(nc, ident)

            xt_sb = big.tile([128, PO, T], BF16, tag="xt")
            kt_sb = big.tile([128, HP, T], BF16, tag="kt")
            qt_sb = big.tile([128, HP, T], BF16, tag="qt")
            v_sb = big.tile([128, NB, HL, 65], BF16, tag="v")
            nc.gpsimd.memset(v_sb[:, :, :, 64:65], 1.0)

            # pre-packed bf16 weights: one contiguous row per partition
            wq_sb = wsb.tile([128, PO, CD], BF16, tag="wq")
            wk_sb = wsb.tile([128, PO, CD], BF16, tag="wk")
            wv_sb = wsb.tile([128, PO, CD], BF16, tag="wv")
            wo_sb = wsb.tile([128, FC, D], BF16, tag="wo")
            nc.scalar.dma_start(wk_sb, wk_d.rearrange("p (a b) -> p a b", a=PO))
            nc.gpsimd.dma_start(wq_sb, wq_d.rearrange("p (a b) -> p a b", a=PO))
            nc.scalar.dma_start(wv_sb, wv_d.rearrange("p (a b) -> p a b", a=PO))
            nc.gpsimd.dma_start(wo_sb, wo_d.rearrange("p (a b) -> p a b", a=FC))

            # interleaved emission: attention quad g follows its projection
            # group; the scheduler fills exp stalls with later projections
            for g in range(4):
                for po in range(PO):
                    nc.sync.dma_start_transpose(
                        xt_sb[:, po, g * 512:(g + 1) * 512],
                        x_d[g * 512:(g + 1) * 512, po * 128:(po + 1) * 128])
                _emit_proj_tw(nc, g, xt_sb, mmp,
                              wq_sb, wk_sb, wv_sb, kt_sb, qt_sb, v_sb)
                qb0 = g * QUAD
                ctxt16s = _emit_quad(nc, qb0, kt_sb, qt_sb, v_sb,
                                     utri_neg, ident,
                                     stp, ptp, cxp, lvp, ctxt16p, dramp)
                for qloc in range(QUAD):
                    _emit_out_proj_qb(nc, qb0, qloc, ctxt16s, wo_sb,
                                      mmp, osbp, out_d)

    nc.compile()
    return nc


_CACHE = {}


def _get_nc():
    if "nc" not in _CACHE:
        _CACHE["nc"] = build_nc()
    return _CACHE["nc"]


def _pack_w(w):
    """[128k, N] -> [128, k*N] bf16: partition p holds rows {k*128+p}."""
    k = w.shape[0] // 128
    n = w.shape[1]
    return np.ascontiguousarray(
        w.reshape(k, 128, n).transpose(1, 0, 2).reshape(128, k * n)
    ).astype(BF16NP)


def make_in_maps(x, Wq, Wk, Wv, Wo):
    x = np.asarray(x, np.float32)
    Wq = np.asarray(Wq, np.float32)
    Wk = np.asarray(Wk, np.float32)
    Wv = np.asarray(Wv, np.float32)
    Wo = np.asarray(Wo, np.float32)
    in_maps = []
    for c in range(8):
        b, hh = c // 2, c % 2
        cols = slice(hh * CD, (hh + 1) * CD)
        in_maps.append({
            "x": np.ascontiguousarray(x[b]).astype(BF16NP),
            "wq": _pack_w(Wq[:, cols]),
            "wk": _pack_w(Wk[:, cols]),
            "wv": _pack_w(Wv[:, cols]),
            "wo": _pack_w(Wo[cols, :]),
        })
    return in_maps


def gather_output(results, bo):
    bo = np.asarray(bo, np.float32)
    out = np.empty((B, T, D), np.float32)
    for b in range(B):
        out[b] = results[2 * b]["out"] + results[2 * b + 1]["out"] + bo[None, :]
    return out


def kernel(x, Wq, Wk, Wv, Wo, bo):
    nc = _get_nc()
    in_maps = make_in_maps(x, Wq, Wk, Wv, Wo)
    res = run_bass_kernel_spmd(nc, in_maps, core_ids=list(range(8)))
    return gather_output(res.results, bo)
